# revision 4
# baseline (speedup 1.0000x reference)
"""Trainium2 Bass kernel for cross "efficient attention".

Reference computation (per batch b, head h, with C=128, HEADS=8, hc=16, n=16384):
    k = x2[b].reshape(HEADS, hc, n); v = x1[b].reshape(HEADS, hc, n)
    key_sm   = softmax(k, axis=-1)          # over n
    query_sm = softmax(k, axis=1)           # over hc (head channels)
    context  = key_sm @ v^T                 # (hc, hc)
    out[b,h] = context^T @ query_sm         # (hc, n)

Sharding: data-parallel over batch B=8 across the 8 NeuronCores (no
collectives).  Inputs are ~N(0,1), so softmax is computed without the
max-subtraction (exp never overflows):
    key_sm   = exp(k) / rowsum,   rowsum[c] = sum_n exp(k[c, n])
    query_sm = exp(k) / colsum,   colsum[h, n] = sum_{c in head h} exp(k[c, n])
    out = (BD^T @ exp(k)) * (1 / (BD8^T @ exp(k)))
where BD is the 128x128 block-diagonal matrix of per-head contexts
(rows scaled by 1/rowsum) and BD8 broadcasts per-head colsums to all 16
channels of the head.
"""

import os
import numpy as np
from contextlib import ExitStack

B, C, H, W = 8, 128, 128, 128
N = H * W                 # 16384
HEADS, HC = 8, 16
NCORES = 8

SLAB = 2048               # load/exp/transpose slab width
NSLAB = N // SLAB         # 8
TS = 512                  # phase-D n-tile width (one PSUM bank of fp32)
NTILES = N // TS          # 32

_cache: dict = {}


def _build():
    import concourse.bass as bass
    import concourse.tile as tile
    from concourse import bacc, mybir

    FP32 = mybir.dt.float32
    BF16 = mybir.dt.bfloat16
    AF = mybir.ActivationFunctionType

    nc = bacc.Bacc("TRN2", target_bir_lowering=False, debug=False)

    x1 = nc.dram_tensor("x1", [C, N], FP32, kind="ExternalInput")
    x2 = nc.dram_tensor("x2", [C, N], FP32, kind="ExternalInput")
    bd8_in = nc.dram_tensor("bd8", [C, C], BF16, kind="ExternalInput")
    out = nc.dram_tensor("out", [C, N], FP32, kind="ExternalOutput")

    with tile.TileContext(nc) as tc:
        with ExitStack() as ctx:
            big = ctx.enter_context(tc.tile_pool(name="big", bufs=1))
            ldp = ctx.enter_context(tc.tile_pool(name="ld", bufs=2))
            smalls = ctx.enter_context(tc.tile_pool(name="smalls", bufs=1))
            rcpool = ctx.enter_context(tc.tile_pool(name="rcp", bufs=3))
            outp = ctx.enter_context(tc.tile_pool(name="outp", bufs=3))
            ps_ctx = ctx.enter_context(tc.tile_pool(name="psctx", bufs=1, space="PSUM"))
            ps_cs = ctx.enter_context(tc.tile_pool(name="pscs", bufs=2, space="PSUM"))
            ps_att = ctx.enter_context(tc.tile_pool(name="psatt", bufs=2, space="PSUM"))

            exp_nat = big.tile([C, N], BF16, tag="exp_nat")
            exp_T = big.tile([C, N], BF16, tag="exp_T")
            vT = big.tile([C, N], BF16, tag="vT")
            x1_bf = big.tile([C, N], BF16, tag="x1_bf")
            rs_acc = smalls.tile([C, NSLAB], FP32, tag="rs_acc")

            # ---- Phase A: load, exp (+rowsum), cast, transposes ----
            for i in range(NSLAB):
                sl = bass.ts(i, SLAB)
                x2t = ldp.tile([C, SLAB], FP32, tag="x2ld")
                nc.sync.dma_start(out=x2t[:], in_=x2[:, sl])
                nc.scalar.activation(
                    exp_nat[:, sl], x2t[:], AF.Exp,
                    accum_out=rs_acc[:, i:i + 1],
                )
                # SWDGE cast-load f32 -> bf16
                nc.gpsimd.dma_start(out=x1_bf[:, sl], in_=x1[:, sl])
                # blocked xbar transposes: out[p, j, c] = in[c, j*128 + p]
                nc.sync.dma_start(
                    out=exp_T[:, sl].rearrange("p (j c) -> p j c", c=C),
                    in_=exp_nat[:, sl],
                    transpose=True,
                )
                nc.sync.dma_start(
                    out=vT[:, sl].rearrange("p (j c) -> p j c", c=C),
                    in_=x1_bf[:, sl],
                    transpose=True,
                )

            # ---- Phase B: context accumulation over all n-chunks ----
            ctx_ps = ps_ctx.tile([C, C], FP32, tag="ctx")
            NCHUNK = N // C   # 128
            for j in range(NCHUNK):
                cs = bass.ts(j, C)
                nc.tensor.matmul(
                    ctx_ps[:],
                    exp_T[:, cs],      # lhsT: (n0=128, c_k=128)
                    vT[:, cs],         # rhs : (n0=128, c_v=128)
                    start=(j == 0),
                    stop=(j == NCHUNK - 1),
                )

            # ---- Phase C: block-diagonal weights ----
            bd8 = smalls.tile([C, C], BF16, tag="bd8")
            nc.sync.dma_start(out=bd8[:], in_=bd8_in[:])

            rowsum = smalls.tile([C, 1], FP32, tag="rowsum")
            nc.vector.tensor_reduce(
                rowsum[:], rs_acc[:], mybir.AxisListType.X, mybir.AluOpType.add
            )
            rs_rcp = smalls.tile([C, 1], FP32, tag="rs_rcp")
            nc.vector.reciprocal(rs_rcp[:], rowsum[:])

            # bd = (ctx * 1/rowsum per row) masked to the diagonal blocks
            scaled = smalls.tile([C, C], BF16, tag="scaled")
            nc.vector.tensor_scalar(
                scaled[:], ctx_ps[:], rs_rcp[:, 0:1], None, mybir.AluOpType.mult
            )
            bd = smalls.tile([C, C], BF16, tag="bd")
            nc.vector.tensor_mul(bd[:], scaled[:], bd8[:])

            # ---- Phase D: per n-tile colsum, reciprocal, attend, store ----
            for t in range(NTILES):
                sl = bass.ts(t, TS)
                cs_ps = ps_cs.tile([C, TS], FP32, tag="cs")
                nc.tensor.matmul(cs_ps[:], bd8[:], exp_nat[:, sl])
                rcp = rcpool.tile([C, TS], FP32, tag="rcp")
                nc.vector.reciprocal_approx_fast(out=rcp[:], in_=cs_ps[:])
                att_ps = ps_att.tile([C, TS], FP32, tag="att")
                nc.tensor.matmul(att_ps[:], bd[:], exp_nat[:, sl])
                ot = outp.tile([C, TS], FP32, tag="ot")
                nc.vector.tensor_mul(ot[:], att_ps[:], rcp[:])
                nc.sync.dma_start(out=out[:, sl], in_=ot[:])

    nc.compile()
    return nc


def _get_nc():
    if "nc" not in _cache:
        _cache["nc"] = _build()
    return _cache["nc"]


def _bd8_np() -> np.ndarray:
    import ml_dtypes

    m = np.zeros((C, C), dtype=np.float32)
    for h in range(HEADS):
        m[h * HC:(h + 1) * HC, h * HC:(h + 1) * HC] = 1.0
    return m.astype(ml_dtypes.bfloat16)


def kernel(x1: np.ndarray, x2: np.ndarray) -> np.ndarray:
    from concourse.bass_utils import run_bass_kernel_spmd

    nc = _get_nc()
    x1 = np.asarray(x1, dtype=np.float32).reshape(B, C, N)
    x2 = np.asarray(x2, dtype=np.float32).reshape(B, C, N)
    bd8 = _bd8_np()
    in_maps = [{"x1": x1[i], "x2": x2[i], "bd8": bd8} for i in range(NCORES)]
    res = run_bass_kernel_spmd(nc, in_maps, core_ids=list(range(NCORES)))
    outs = [res.results[i]["out"] for i in range(NCORES)]
    return np.stack(outs, axis=0).reshape(B, C, H, W).astype(np.float32)
